# revision 7
# baseline (speedup 1.0000x reference)
"""GCN block (GraphConv + residual + BatchNorm + ReLU) on 8 TRN2 NeuronCores.

Strategy (graph/data parallel per sharding hint): destination nodes are
partitioned into 8*G groups of <=SL nodes, balanced by in-degree so every
group has <=T*128 incident edges. x is uploaded once as bf16 row-shards
(N/8 rows per core) and AllGathered on-device into a full [N+128, D] table
(last 128 rows zero, the gather target for padded slots). Each core handles
G groups, processed in blocks of 4 so PSUM banks are used full-width (512):
  - indirect-DMA gathers source rows x[src] for each 128-edge tile
  - a one-hot "selection" matmul segment-sums edge tiles into PSUM quarters,
    producing agg^T [feat, dst] per group (degree norms folded into the
    selection entries); selection tiles for 156 edge-tiles at a time are
    built with two broadcast-AP vector ops
  - agg^T @ W in one 512-wide matmul per block; residual rows are
    indirect-gathered from the same table and added into the same PSUM bank
    via matmuls against the identity (computes x^T)
  - BN batch stats accumulated per-feature during the PSUM->SBUF copies,
    AllReduce'd across the 8 cores, then fused scale/shift+ReLU, transpose
    back (again matmul against identity) and a single rearranged DMA per
    block stores 4 groups of SL=121 rows as bf16.
The bias b cancels against the batch mean and is dropped. Host does only
graph-structure preprocessing (degrees, balanced grouping, edge slotting -
all vectorized) plus the shard/unshard permutations. Edge src index (17b)
and dst slot (7b) travel packed in one int32 and are unpacked on device.
All bulk tensors move as bf16; accumulation stays fp32 in PSUM.
"""
import hashlib
import numpy as np
import ml_dtypes

BF16 = ml_dtypes.bfloat16

N, D, E = 100000, 128, 600000
EPS = 1e-5
NCORES = 8
P = 128
G = 104          # dst groups per core (26 blocks of 4)
T = 6            # 128-edge tiles per group
GT = G * T
SL = 121         # used slots per group (snake assign guarantees <= ceil(N/(8G)))
NSH = N // NCORES
SELC = 156       # edge tiles per selection-build chunk (4 chunks of 156 = GT)

_cache = {}


def _build_nc():
    import concourse.bass as bass
    import concourse.bacc as bacc
    import concourse.mybir as mybir
    import concourse.tile as tile
    from concourse.masks import make_identity

    f32 = mybir.dt.float32
    bf16 = mybir.dt.bfloat16
    i32 = mybir.dt.int32
    nc = bacc.Bacc(None, target_bir_lowering=False, debug=False)
    xs = nc.declare_dram_parameter("xs", [NSH, D], bf16, isOutput=False)
    pidx = nc.declare_dram_parameter("pidx", [P, GT], i32, isOutput=False)
    wgt = nc.declare_dram_parameter("wgt", [P, GT], bf16, isOutput=False)
    idxr = nc.declare_dram_parameter("idxr", [P, G], i32, isOutput=False)
    gb = nc.declare_dram_parameter("gb", [P, 2], f32, isOutput=False)
    wm = nc.declare_dram_parameter("wm", [D, D], bf16, isOutput=False)
    y = nc.declare_dram_parameter("y", [G * SL, D], bf16, isOutput=True)

    AF = mybir.ActivationFunctionType
    OP = mybir.AluOpType
    NBLK = G // 4

    with tile.TileContext(nc) as tc:
        with tc.tile_pool(name="const", bufs=1) as cb, \
             tc.tile_pool(name="big", bufs=1) as bigp, \
             tc.tile_pool(name="sel", bufs=2) as selp, \
             tc.tile_pool(name="rows", bufs=6) as rowsp, \
             tc.tile_pool(name="xr", bufs=4) as xrp, \
             tc.tile_pool(name="aggs", bufs=2) as aggsp, \
             tc.tile_pool(name="scr", bufs=2) as scrp, \
             tc.tile_pool(name="ob", bufs=2) as obp, \
             tc.tile_pool(name="pa", bufs=2, space="PSUM") as pa, \
             tc.tile_pool(name="pz", bufs=2, space="PSUM") as pz, \
             tc.tile_pool(name="po", bufs=2, space="PSUM") as po, \
             tc.tile_pool(name="dram", bufs=1, space="DRAM") as dram:

            # ---- AllGather the x shard into a full bf16 table, +128 zero rows
            xg = dram.tile([N + P, D], bf16)
            cin = dram.tile([NSH, D], bf16)
            nc.sync.dma_start(out=cin[:], in_=xs[:])
            nc.gpsimd.collective_compute(
                "AllGather", OP.bypass,
                replica_groups=[list(range(NCORES))],
                ins=[cin[:]], outs=[xg[0:N, :]],
            )
            zrow = cb.tile([P, D], bf16)
            nc.gpsimd.memset(zrow[:], 0.0)
            nc.sync.dma_start(out=xg[N:N + P, :], in_=zrow[:])

            # ---- constants + packed index decode
            pidx_sb = cb.tile([P, GT], i32)
            nc.sync.dma_start(out=pidx_sb[:], in_=pidx[:])
            idx_sb = cb.tile([P, GT], i32)
            nc.vector.tensor_scalar(out=idx_sb[:], in0=pidx_sb[:],
                                    scalar1=0x1FFFF, scalar2=None,
                                    op0=OP.bitwise_and)
            dslt_i = cb.tile([P, GT], i32)
            nc.vector.tensor_scalar(out=dslt_i[:], in0=pidx_sb[:],
                                    scalar1=17, scalar2=None,
                                    op0=OP.logical_shift_right)
            dsl_sb = cb.tile([P, GT], f32)
            nc.vector.tensor_copy(out=dsl_sb[:], in_=dslt_i[:])
            w_edge = cb.tile([P, GT], bf16)
            nc.sync.dma_start(out=w_edge[:], in_=wgt[:])
            idxr_sb = cb.tile([P, G], i32)
            nc.sync.dma_start(out=idxr_sb[:], in_=idxr[:])
            w_sb = cb.tile([D, D], bf16)
            nc.sync.dma_start(out=w_sb[:], in_=wm[:])
            gb_sb = cb.tile([P, 2], f32)
            nc.sync.dma_start(out=gb_sb[:], in_=gb[:])
            iota_sb = cb.tile([P, P], f32)
            nc.gpsimd.iota(iota_sb[:], pattern=[[1, P]], channel_multiplier=0,
                           allow_small_or_imprecise_dtypes=True)
            ident = cb.tile([P, P], bf16)
            make_identity(nc, ident[:])

            hT = bigp.tile([P, G * P], bf16)
            s1all = bigp.tile([P, NBLK], f32)
            s2all = bigp.tile([P, NBLK], f32)

            # ---- selection chunks (built lazily before first use)
            sel_tiles = {}

            def build_sel_chunk(k):
                c0 = k * SELC
                sel = selp.tile([P, SELC, P], bf16, tag="sel")
                nc.vector.tensor_tensor(
                    out=sel[:],
                    in0=dsl_sb[:, c0:c0 + SELC].to_broadcast([P, SELC, P]),
                    in1=iota_sb[:].rearrange("p d -> p () d").to_broadcast([P, SELC, P]),
                    op=OP.is_equal)
                nc.vector.tensor_tensor(
                    out=sel[:], in0=sel[:],
                    in1=w_edge[:, c0:c0 + SELC].to_broadcast([P, SELC, P]),
                    op=OP.mult)
                sel_tiles[k] = sel

            # ---- main loop: blocks of 4 groups
            for b in range(NBLK):
                aggp = pa.tile([P, 4 * P], f32, tag="agg", space="PSUM")
                for q in range(4):
                    g = 4 * b + q
                    for t in range(T):
                        c = g * T + t
                        k, kc = divmod(c, SELC)
                        if k not in sel_tiles:
                            build_sel_chunk(k)
                        rows = rowsp.tile([P, D], bf16, tag="rows")
                        nc.gpsimd.indirect_dma_start(
                            out=rows[:], out_offset=None, in_=xg[:],
                            in_offset=bass.IndirectOffsetOnAxis(
                                ap=idx_sb[:, c:c + 1], axis=0),
                        )
                        nc.tensor.matmul(out=aggp[:, q * P:(q + 1) * P],
                                         lhsT=rows[:], rhs=sel_tiles[k][:, kc, :],
                                         start=(t == 0), stop=(t == T - 1),
                                         skip_group_check=True)
                aggs = aggsp.tile([P, 4 * P], bf16, tag="aggs")
                nc.scalar.copy(out=aggs[:], in_=aggp[:])
                zp = pz.tile([P, 4 * P], f32, tag="z", space="PSUM")
                nc.tensor.matmul(out=zp[:], lhsT=w_sb[:], rhs=aggs[:],
                                 start=True, stop=False, skip_group_check=True)
                for q in range(4):
                    g = 4 * b + q
                    xtile = xrp.tile([P, D], bf16, tag="xr")
                    nc.gpsimd.indirect_dma_start(
                        out=xtile[:], out_offset=None, in_=xg[:],
                        in_offset=bass.IndirectOffsetOnAxis(
                            ap=idxr_sb[:, g:g + 1], axis=0),
                    )
                    nc.tensor.matmul(out=zp[:, q * P:(q + 1) * P],
                                     lhsT=xtile[:], rhs=ident[:],
                                     start=False, stop=True, skip_group_check=True)
                hsl = hT[:, b * 4 * P:(b + 1) * 4 * P]
                nc.scalar.activation(out=hsl, in_=zp[:], func=AF.Identity,
                                     accum_out=s1all[:, b:b + 1])
                sq = scrp.tile([P, 4 * P], bf16, tag="sq")
                nc.scalar.activation(out=sq[:], in_=hsl, func=AF.Square,
                                     accum_out=s2all[:, b:b + 1])

            # ---- BN stats reduce + AllReduce across cores
            stats = cb.tile([P, 2], f32)
            nc.vector.reduce_sum(out=stats[:, 0:1], in_=s1all[:], axis=mybir.AxisListType.X)
            nc.vector.reduce_sum(out=stats[:, 1:2], in_=s2all[:], axis=mybir.AxisListType.X)
            sin = dram.tile([P, 2], f32)
            sout = dram.tile([P, 2], f32)
            nc.gpsimd.dma_start(out=sin[:], in_=stats[:])
            nc.gpsimd.collective_compute(
                "AllReduce", OP.add,
                replica_groups=[list(range(NCORES))],
                ins=[sin.opt()], outs=[sout.opt()],
            )
            red = cb.tile([P, 2], f32)
            nc.gpsimd.dma_start(out=red[:], in_=sout[:])

            mean = cb.tile([P, 1], f32)
            nc.scalar.mul(out=mean[:], in_=red[:, 0:1], mul=1.0 / N)
            ex2 = cb.tile([P, 1], f32)
            nc.scalar.mul(out=ex2[:], in_=red[:, 1:2], mul=1.0 / N)
            msq = cb.tile([P, 1], f32)
            nc.scalar.activation(out=msq[:], in_=mean[:], func=AF.Square)
            var = cb.tile([P, 1], f32)
            nc.vector.tensor_tensor(out=var[:], in0=ex2[:], in1=msq[:],
                                    op=OP.subtract)
            epsc = cb.tile([P, 1], f32)
            nc.gpsimd.memset(epsc[:], EPS)
            std = cb.tile([P, 1], f32)
            nc.scalar.activation(out=std[:], in_=var[:], func=AF.Sqrt, bias=epsc[:])
            rstd = cb.tile([P, 1], f32)
            nc.vector.reciprocal(out=rstd[:], in_=std[:])
            scale = cb.tile([P, 1], f32)
            nc.vector.tensor_tensor(out=scale[:], in0=rstd[:], in1=gb_sb[:, 0:1],
                                    op=OP.mult)
            mscl = cb.tile([P, 1], f32)
            nc.vector.tensor_tensor(out=mscl[:], in0=mean[:], in1=scale[:],
                                    op=OP.mult)
            shift = cb.tile([P, 1], f32)
            nc.vector.tensor_tensor(out=shift[:], in0=gb_sb[:, 1:2], in1=mscl[:],
                                    op=OP.subtract)

            # ---- normalize + relu + transpose back + store (4 groups/DMA)
            for b in range(NBLK):
                ot = obp.tile([P, 4 * P], bf16, tag="ot")
                nc.scalar.activation(out=ot[:], in_=hT[:, b * 4 * P:(b + 1) * 4 * P],
                                     func=AF.Relu, scale=scale[:], bias=shift[:])
                otp = po.tile([P, 4 * P], f32, tag="o", space="PSUM")
                for q in range(4):
                    nc.tensor.matmul(out=otp[:, q * P:(q + 1) * P],
                                     lhsT=ot[:, q * P:(q + 1) * P], rhs=ident[:],
                                     start=True, stop=True, skip_group_check=True)
                ob = obp.tile([P, 4 * P], bf16, tag="obf")
                nc.vector.tensor_copy(out=ob[:], in_=otp[:])
                nc.sync.dma_start(
                    out=y[4 * b * SL:(4 * b + 4) * SL, :].rearrange(
                        "(q s) f -> s q f", q=4),
                    in_=ob[0:SL, :].rearrange("p (q f) -> p q f", q=4))

    nc.compile()
    return nc


def _snake_assign(deg_in):
    """Assign nodes to NCORES*G groups, balanced by in-degree: nodes sorted by
    degree desc are dealt boustrophedon across groups. Slot of a node is its
    deal round, so slots stay < ceil(N / ngroups) <= SL."""
    ngroups = NCORES * G
    order = np.argsort(-deg_in, kind="stable")
    i = np.arange(N)
    r = i // ngroups
    pos = i % ngroups
    grp = np.where(r % 2 == 0, pos, ngroups - 1 - pos).astype(np.int32)
    node_group = np.empty(N, np.int32)
    node_slot = np.empty(N, np.int32)
    node_group[order] = grp
    node_slot[order] = r.astype(np.int32)
    return node_group, node_slot


def _greedy_assign(deg_in):
    """Fallback LPT greedy (slow Python heap) used only if the snake
    assignment violates the per-group capacity bounds."""
    import heapq
    ngroups = NCORES * G
    order = np.argsort(-deg_in, kind="stable")
    heap = [(0.0, 0, gi) for gi in range(ngroups)]
    heapq.heapify(heap)
    node_group = np.empty(N, np.int32)
    node_slot = np.empty(N, np.int32)
    counts = np.zeros(ngroups, np.int32)
    loads = np.zeros(ngroups, np.int64)
    for node in order:
        while True:
            load, cnt, gi = heapq.heappop(heap)
            if cnt == counts[gi] and load == loads[gi]:
                break
        node_group[node] = gi
        node_slot[node] = counts[gi]
        counts[gi] += 1
        loads[gi] += int(deg_in[node])
        if counts[gi] < SL:
            heapq.heappush(heap, (float(loads[gi]), int(counts[gi]), gi))
    return node_group, node_slot


def _preprocess(edge_index):
    """Host graph-structure preprocessing: degrees, balanced dst grouping,
    per-slot packed src/dslot, edge weights (fully vectorized numpy)."""
    src = np.ascontiguousarray(edge_index[0]).astype(np.int64)
    dst = np.ascontiguousarray(edge_index[1]).astype(np.int64)
    deg_out = np.bincount(src, minlength=N)
    deg_in = np.bincount(dst, minlength=N)
    w_edge = (1.0 / np.sqrt(np.maximum(deg_out[src], 1.0) *
                            np.maximum(deg_in[dst], 1.0))).astype(np.float32)

    ngroups = NCORES * G
    node_group, node_slot = _snake_assign(deg_in)
    loads = np.bincount(node_group[dst], minlength=ngroups)
    counts = np.bincount(node_group, minlength=ngroups)
    if loads.max() > T * P or counts.max() > SL:
        node_group, node_slot = _greedy_assign(deg_in)
        loads = np.bincount(node_group[dst], minlength=ngroups)
        counts = np.bincount(node_group, minlength=ngroups)
    assert loads.max() <= T * P, f"group overload {loads.max()}"
    assert counts.max() <= SL, f"group overfull {counts.max()}"

    # per-edge slot assignment: edges of group gi fill slots sequentially
    egroup = node_group[dst]
    eorder = np.argsort(egroup, kind="stable")
    gstart = np.zeros(ngroups + 1, np.int64)
    np.cumsum(np.bincount(egroup, minlength=ngroups), out=gstart[1:])
    eg = egroup[eorder]
    k = np.arange(E) - gstart[eg]
    t = (k // P).astype(np.int64)
    p = (k % P).astype(np.int64)
    core_e = eg // G
    col = (eg % G) * T + t

    # packed = src | dslot<<17 ; padded slots keep src 0 / dslot 0 / weight 0
    pidx_all = np.zeros((NCORES, P, GT), np.int32)
    wgt_all = np.zeros((NCORES, P, GT), BF16)
    packed = (src[eorder] | (node_slot[dst[eorder]].astype(np.int64) << 17)).astype(np.int32)
    pidx_all[core_e, p, col] = packed
    wgt_all[core_e, p, col] = w_edge[eorder].astype(BF16)

    # residual gather indices: row slot, col g holds the node id, padded -> N
    # (the zero rows appended after the AllGathered table)
    idxr_all = np.full((NCORES, P, G), N, np.int32)
    core_n = node_group // G
    g_n = node_group % G
    idxr_all[core_n, node_slot, g_n] = np.arange(N, dtype=np.int32)

    return node_group, node_slot, pidx_all, wgt_all, idxr_all


def kernel(x, edge_index, W, b, gamma, beta):
    x = np.ascontiguousarray(np.asarray(x, np.float32))
    W = np.asarray(W, np.float32)
    gamma = np.asarray(gamma, np.float32)
    beta = np.asarray(beta, np.float32)

    ekey = hashlib.blake2b(np.ascontiguousarray(edge_index).tobytes(),
                           digest_size=16).hexdigest()
    if _cache.get("ekey") != ekey:
        _cache["prep"] = _preprocess(edge_index)
        _cache["ekey"] = ekey
    node_group, node_slot, pidx_all, wgt_all, idxr_all = _cache["prep"]

    xb = x.astype(BF16)
    wb = W.astype(BF16)
    gb_host = np.stack([gamma, beta], axis=1).astype(np.float32)  # [128,2]
    in_maps = []
    for core in range(NCORES):
        in_maps.append(dict(xs=xb[core * NSH:(core + 1) * NSH],
                            pidx=pidx_all[core], wgt=wgt_all[core],
                            idxr=idxr_all[core], gb=gb_host, wm=wb))

    if "nc" not in _cache:
        _cache["nc"] = _build_nc()
    from concourse.bass_utils import run_bass_kernel_spmd
    import time
    t0 = time.perf_counter()
    res = run_bass_kernel_spmd(_cache["nc"], in_maps, core_ids=list(range(NCORES)))
    _cache["last_wall_s"] = time.perf_counter() - t0

    ybig = np.stack([res.results[core]["y"] for core in range(NCORES)])
    rows = (node_group % G).astype(np.int64) * SL + node_slot
    out = ybig[node_group // G, rows, :].astype(np.float32)
    return out
